# revision 50
# baseline (speedup 1.0000x reference)
"""Trainium2 Bass kernel for the DART masked-MLP + log-semiring chain model.

Computes, for B=8192 samples distributed over 8 NeuronCores (1024 each):
  h1 = relu(x @ (m0*W0).T + b0)
  h2 = relu(h1 @ (m1*W1).T + b1)
  h3 = relu(h2 @ (m2*W2).T + b2)
  theta = (h3 @ (m3*W3).T + b3) -> (B, 128, 2, 4, 4) = (mu, alpha)
  logp  = -0.5*((x - mu)*exp(-alpha))**2 - alpha - 0.5*log(2pi) - log(4)
  out   = logexpmm(first, logexpmm(chain(inner), last))   # (B, 1, 1)

Device strategy (per core):
  - MADE masks premultiplied into the weights host-side; hidden units are
    degree-sorted so the masked weights become block lower triangular and
    ~44% of contraction chunks are skipped.  All four matmul layers run in
    fp8-e4m3 DoubleRow (0.5 PE cycles/row); L1 zero-pads the second
    contraction lane, and all biases are injected with K=1 fp8 DR matmuls
    against constant lanes (no separate bias adds on the vector engines).
  - Software-pipelined wavefront: iteration t emits L1 chunk-pair t, L2
    pair t-1, L3 pair t-2, L4 chunk t-3 matmuls + logp front (et/dt/tt/sq),
    logp back (p1/P) for t-4, and chain-fold block t-4.  Every cross-engine
    dependency is a full iteration old, so the in-order engine streams never
    stall on each other, and the serial fold chain starts ~wave 0.
  - The log-semiring product is folded in the LINEAR domain from both ends
    as vector folds (one DVE op pair per position, bf16, renormalized every
    16 positions with Ln bookkeeping on ACT off the critical path; the
    measured worst 16-position decay e^-62 is far above the bf16 floor).
    Meet point at position 112: the left fold (111 steps) rides the waves;
    only the 15-step right fold trails the last wave, zipped with the left
    fold's final block so the two serial chains hide each other's gaps.
  - Left-chunk W3/b3 columns are packed (pos,t,b,a)-transposed host-side so
    every logp stage and P store stays stride-1 packed (DVE 2x bf16 mode).
  - Engine assignment: relu + the two exps on ACT, dt/P/folds on DVE,
    tt/sq muls on Pool (gpsimd; it cannot read PSUM), matmuls on PE.
    Weights stream as a few 128-descriptor whole-block DMAs ordered by
    first use (per-chunk DMAs previously saturated HWDGE descriptor gen).
"""

import math

import numpy as np
import ml_dtypes

I = 128          # input size / positions
H = 2048         # hidden
A = 4            # alpha_dim
K = 2 * A * A    # 32 theta entries per position
B = 8192
NCORES = 8
BL = B // NCORES          # 1024 samples per core
NG = BL // 128            # 8 sample groups of 128
NK = H // 128             # 16 hidden chunks
NQ = (I * K) // 512       # 8 output q-chunks (512 wide = 16 positions)
C0 = 0.5 * math.log(2.0 * math.pi) + math.log(4.0)
SW = 13                   # weight scale 2^SW for fp8
SA = 5                    # activation scale 2^SA for fp8
SW0 = 11                  # L1 weight scale (|w0| <= 1/sqrt(128))
SAX = 5                   # x scale for the fp8 L1 input

# relu engine split per layer: "A" (ACT fused) or "D" (DVE tensor_scalar
# after a K=1 bias matmul; Pool cannot read PSUM).  L1's early chunks go to
# DVE, which idles during pipeline fill; L2/L3 relus stay on ACT.
RELU_MAP_L = [["A"] * 16, ["A"] * 16, ["A"] * 16]

_DCH = [m for m in range(NK) if RELU_MAP_L[0][m] == "D"]

_bf16 = ml_dtypes.bfloat16
_f8 = ml_dtypes.float8_e4m3


def _make_meta():
    hdeg = np.arange(H) % (I - 1)
    perm = np.argsort(hdeg, kind="stable")
    sdeg = hdeg[perm]
    km_l = []
    for m in range(NK):
        dhi = sdeg[128 * m + 127]
        km_l.append(max(k for k in range(NK) if sdeg[128 * k] <= dhi))
    km_4 = []
    for qc in range(NQ):
        dhi = 16 * qc + 15 - 1
        cands = [k for k in range(NK) if sdeg[128 * k] <= dhi]
        km_4.append(max(cands) if cands else -1)
    return perm, km_l, km_4


_PERM, _KM_L, _KM_4 = _make_meta()
_NP_L = [k // 2 + 1 for k in _KM_L]     # fp8 DoubleRow pair-chunks per m
_NP_4 = [k // 2 + 1 for k in _KM_4]     # per L4 q-chunk
_OFF1 = np.cumsum([0] + [p * 256 for p in _NP_L]).tolist()
_OFF3 = np.cumsum([0] + [p * 1024 for p in _NP_4]).tolist()


def _prep_inputs(x, W0, b0, W1, b1, W2, b2, W3, b3):
    """Host-side: premask, degree-sort, pack and cast the weights."""
    inp = np.arange(I)
    degrees = [inp] + [np.arange(H) % (I - 1) for _ in range(3)] + [np.arange(I) - 1]
    masks = [
        (d1[:, None] >= d0[None, :]).astype(np.float32)
        for d0, d1 in zip(degrees[:-1], degrees[1:])
    ]
    masks[-1] = np.repeat(masks[-1], K, axis=0)

    p = _PERM
    W0s = (masks[0] * W0)[p]
    b0s = b0[p]
    W1s = (masks[1] * W1)[p][:, p]
    b1s = b1[p]
    W2s = (masks[2] * W2)[p][:, p]
    b2s = b2[p]
    W3s = (masks[3] * W3)[:, p]

    # Left chunks (qc 0-3): transpose (a,b)->(b,a) within each position so P
    # lands b-major (fold-ready) with stride-1 stores.  e = i*32 + t*16 + a*4 + b.
    colperm = np.arange(I * K)
    e = colperm.reshape(I, 2, A, A)
    e_t = np.transpose(e, (0, 1, 3, 2)).reshape(I * K)
    left_cols = np.arange(0, 4 * 512)
    colperm[left_cols] = e_t[left_cols]
    W3p = W3s[colperm]
    b3p = b3[colperm]

    # L1 weights as fp8 DoubleRow pairs: lane 0 = w0*2^SW0, lane 1 = 0.
    # x is cast to fp8 (x*2^SAX) on chip; psum = h1pre * 2^(SW0+SAX).
    w0dr = np.zeros((I, 2, H), np.float32)
    w0dr[:, 0, :] = W0s.T * float(2 ** SW0)
    assert np.abs(w0dr).max() < 440.0
    w0t = np.ascontiguousarray(w0dr.reshape(I, 2 * H)).astype(_f8)

    def pack_pairs(WT, npairs, out_w, col_starts):
        cols = []
        for blk, (np_, c0) in enumerate(zip(npairs, col_starts)):
            for kp in range(np_):
                blkdat = np.stack(
                    [WT[256 * kp + 128 * par:256 * kp + 128 * (par + 1),
                        c0:c0 + out_w] for par in range(2)], axis=1)
                cols.append(blkdat.reshape(128, 2 * out_w))
        arr = np.concatenate(cols, axis=1) * float(2 ** SW)
        assert np.abs(arr).max() < 440.0, np.abs(arr).max()
        return np.ascontiguousarray(arr).astype(_f8)

    w1t = pack_pairs(W1s.T, _NP_L, 128, [128 * m for m in range(NK)])
    w2t = pack_pairs(W2s.T, _NP_L, 128, [128 * m for m in range(NK)])
    w3t = pack_pairs(W3p.T, _NP_4, 512, [512 * q for q in range(NQ)])

    # per-partition biases for ACT relu chunks (scaled 2^SA)
    b0r = np.ascontiguousarray(b0s.reshape(NK, 128).T
                               ).astype(np.float32) * float(2 ** SA)
    b0q = np.ascontiguousarray(b0s.reshape(NK, 128).T
                               ).astype(np.float32) * float(2 ** (SW0 + SAX))
    b1r = np.ascontiguousarray(b1s.reshape(NK, 128).T
                               ).astype(np.float32) * float(2 ** SA)
    b2r = np.ascontiguousarray(b2s.reshape(NK, 128).T
                               ).astype(np.float32) * float(2 ** SA)
    # bias rows for DVE relu chunks (bias-matmul into PSUM domain), packed
    # to just the D chunks' columns
    if _DCH:
        dsel = np.concatenate([np.arange(128 * m, 128 * (m + 1))
                               for m in _DCH])
        b0w = (b0s[None, dsel] * float(2 ** (SW0 + SAX))).astype(_bf16)
    else:
        b0w = np.zeros((1, 1), np.float32).astype(_bf16)
    b1w = np.zeros((1, 1), np.float32).astype(_bf16)                  # unused
    b2w = np.zeros((1, 1), np.float32).astype(_bf16)                  # unused
    # L4 bias as fp8 K=1 DoubleRow rows: per chunk [1024] = (b3*2^SW | zeros);
    # the stationary constant lane supplies 2^SA, so psum += b3*2^(SW+SA).
    b3blk = np.zeros((NQ, 2, 512), np.float32)
    b3blk[:, 0, :] = b3p.reshape(NQ, 512) * float(2 ** SW)
    assert np.abs(b3blk).max() < 440.0
    b3r = np.ascontiguousarray(b3blk.reshape(1, NQ * 1024)).astype(_f8)

    common = dict(w0t=w0t, w1t=w1t, w2t=w2t, w3t=w3t,
                  b0r=b0r, b0q=b0q, b1r=b1r, b2r=b2r,
                  b0w=b0w, b1w=b1w, b2w=b2w, b3r=b3r)
    in_maps = []
    for c in range(NCORES):
        m = dict(common)
        m["x"] = np.ascontiguousarray(x[c * BL:(c + 1) * BL]).astype(np.float32)
        in_maps.append(m)
    return in_maps


_NC_CACHE = {}
_MARKERS = {}


def _patch_ldw_opt():
    """Enable walrus's ldweights overlap: without it every matmul pays a
    serial ~107ns stationary load (measured ~310ns/DoubleRow-matmul vs the
    ~107ns model), which makes instruction count the real PE wall."""
    import concourse.bass_utils as _bu
    if getattr(_bu, "_ldw_patched", False):
        return
    _orig = _bu.run_command

    def _patched(cmd, *a, **k):
        if isinstance(cmd, list):
            cmd = ["--enable-ldw-opt=true" if c == "--enable-ldw-opt=false"
                   else c for c in cmd]
        return _orig(cmd, *a, **k)

    _bu.run_command = _patched
    _bu._ldw_patched = True


def _build_nc(loop_reps=1):
    import concourse.bacc as bacc
    import concourse.tile as tile
    import concourse.mybir as mybir
    from concourse.masks import make_identity
    from contextlib import ExitStack

    f32 = mybir.dt.float32
    bf16 = mybir.dt.bfloat16
    f8 = mybir.dt.float8e4
    AF = mybir.ActivationFunctionType
    ALU = mybir.AluOpType
    AX = mybir.AxisListType

    nc = bacc.Bacc("TRN2")
    x_d = nc.declare_dram_parameter("x", [BL, I], f32, isOutput=False)
    w0_d = nc.declare_dram_parameter("w0t", [I, 2 * H], f8, isOutput=False)
    w1_d = nc.declare_dram_parameter("w1t", [128, _OFF1[-1]], f8, isOutput=False)
    w2_d = nc.declare_dram_parameter("w2t", [128, _OFF1[-1]], f8, isOutput=False)
    w3_d = nc.declare_dram_parameter("w3t", [128, _OFF3[-1]], f8, isOutput=False)
    b0_d = nc.declare_dram_parameter("b0r", [128, NK], f32, isOutput=False)
    b0q_d = nc.declare_dram_parameter("b0q", [128, NK], f32, isOutput=False)
    b1_d = nc.declare_dram_parameter("b1r", [128, NK], f32, isOutput=False)
    b2_d = nc.declare_dram_parameter("b2r", [128, NK], f32, isOutput=False)
    b0w_d = nc.declare_dram_parameter("b0w", [1, max(1, len(_DCH) * 128)], bf16, isOutput=False)
    b1w_d = nc.declare_dram_parameter("b1w", [1, 1], bf16, isOutput=False)
    b2w_d = nc.declare_dram_parameter("b2w", [1, 1], bf16, isOutput=False)
    b3_d = nc.declare_dram_parameter("b3r", [1, NQ * 1024], f8, isOutput=False)
    out_d = nc.declare_dram_parameter("out", [128, NG], f32, isOutput=True)

    SDESC = float(2.0 ** (-(SW + SA)))

    with ExitStack() as ctx:
        tc = ctx.enter_context(tile.TileContext(nc))
        consts = ctx.enter_context(tc.tile_pool(name="consts", bufs=1))
        a13p = ctx.enter_context(tc.tile_pool(name="a13p", bufs=NK // 2))
        a2p = ctx.enter_context(tc.tile_pool(name="a2p", bufs=NK // 2))
        a3p = ctx.enter_context(tc.tile_pool(name="a3p", bufs=NK // 2))
        pallpool = ctx.enter_context(tc.tile_pool(name="pallp", bufs=8))
        ltmp = ctx.enter_context(tc.tile_pool(name="ltmp", bufs=8))
        ltmp2 = ctx.enter_context(tc.tile_pool(name="ltmp2", bufs=4))
        chpool = ctx.enter_context(tc.tile_pool(name="ch", bufs=4))

        if loop_reps > 1:
            ctx.enter_context(tc.For_i(0, loop_reps, 1))

        def marker(tag):
            _MARKERS[tag] = int(nc.get_next_instruction_name()[2:])

        # ---- constants ----
        xf = consts.tile([128, NG, I], f32)         # x[p, g, i] = x[g*128+p, i]
        nc.sync.dma_start(out=xf, in_=x_d[:, :].rearrange("(g p) i -> p g i", p=128))
        w0sb = consts.tile([128, 2 * H], f8)
        nc.sync.dma_start(out=w0sb, in_=w0_d[:, :])
        b0sb = consts.tile([128, NK], f32)
        nc.sync.dma_start(out=b0sb, in_=b0_d[:, :])
        b0qsb = consts.tile([128, NK], f32)
        nc.sync.dma_start(out=b0qsb, in_=b0q_d[:, :])
        b1sb = consts.tile([128, NK], f32)
        nc.sync.dma_start(out=b1sb, in_=b1_d[:, :])
        b2sb = consts.tile([128, NK], f32)
        nc.sync.dma_start(out=b2sb, in_=b2_d[:, :])
        b3sb = consts.tile([1, NQ * 1024], f8)
        nc.sync.dma_start(out=b3sb, in_=b3_d[:, :])
        cone8 = consts.tile([1, 256], f8)
        nc.vector.memset(cone8, 0.0)
        nc.vector.memset(cone8[0:1, 0:128], float(2 ** SA))
        b0wsb = consts.tile([1, max(1, len(_DCH) * 128)], bf16)
        nc.sync.dma_start(out=b0wsb, in_=b0w_d[:, :])

        onesbl = consts.tile([1, BL], bf16)
        nc.vector.memset(onesbl, 1.0)
        # weight loads in a few large pieces (128 descriptors each), ordered
        # by first use so compute starts as soon as its piece lands
        w1sb = consts.tile([128, _OFF1[-1]], f8)
        w2sb = consts.tile([128, _OFF1[-1]], f8)
        w3sbL = consts.tile([128, _OFF3[4]], f8)
        w3sbR = consts.tile([128, _OFF3[-1] - _OFF3[4]], f8)
        for k in range(NK // 2):
            nc.sync.dma_start(out=w1sb[:, _OFF1[2 * k]:_OFF1[2 * k + 2]],
                              in_=w1_d[:, _OFF1[2 * k]:_OFF1[2 * k + 2]])
            nc.sync.dma_start(out=w2sb[:, _OFF1[2 * k]:_OFF1[2 * k + 2]],
                              in_=w2_d[:, _OFF1[2 * k]:_OFF1[2 * k + 2]])
            if k < 4:
                nc.sync.dma_start(out=w3sbL[:, _OFF3[k]:_OFF3[k + 1]],
                                  in_=w3_d[:, _OFF3[k]:_OFF3[k + 1]])
            else:
                nc.sync.dma_start(
                    out=w3sbR[:, _OFF3[k] - _OFF3[4]:_OFF3[k + 1] - _OFF3[4]],
                    in_=w3_d[:, _OFF3[k]:_OFF3[k + 1]])
        ones1 = consts.tile([1, 128], bf16)
        nc.vector.memset(ones1, 1.0)
        czero = consts.tile([128, 1], f32)
        nc.vector.memset(czero, 0.0)
        nc.const_aps.aps[(f32, 0.0)] = czero[:, :]
        cnegc = consts.tile([128, 1], f32)
        nc.vector.memset(cnegc, -C0)
        ident = consts.tile([128, 128], f32)
        make_identity(nc, ident[:, :])
        NREN = 16
        logs = consts.tile([128, NG, 2, NREN], f32)
        nc.vector.memset(logs, 0.0)
        xt = consts.tile([128, 2, BL], f8)          # xT[i, lane, g*128+b]
        nc.vector.memset(xt, 0.0)

        # P tiles: [128, g, pos, 16e]; left (w 0-3) b-major, right (w 4-7)
        # a-major.  bf16.
        PP = {}
        for w in range(8):
            PP[w] = pallpool.tile([128, NG, 16, 16], bf16, tag="pall",
                                  name=f"pp_{w}")

        # both folds run whole-NG on DVE (Pool free-axis reduce unsupported);
        # the lane machinery is kept for experiments
        LANES = {"L": [("D", 0, NG)], "R": [("D", 0, NG)]}
        state = {("u", f, ln): None for f in "LR" for ln, _, _ in LANES[f]}
        state.update({("n", f, ln): 0 for f in "LR" for ln, _, _ in LANES[f]})

        def renorm(fold, lane, g0, g1, tag):
            # rescale fold vector so max=1; Ln bookkeeping on ACT (parallel)
            f = 0 if fold == "L" else 1
            idx = state[("n", fold, lane)]
            state[("n", fold, lane)] += 1
            u = state[("u", fold, lane)]
            ng = g1 - g0
            eng = nc.vector if lane == "D" else nc.gpsimd
            m16 = chpool.tile([128, ng], f32, tag="m16", name=f"m16_{tag}")
            eng.tensor_reduce(m16[:, :], u[:, :, :], axis=AX.X, op=ALU.max)
            nc.scalar.activation(logs[:, g0:g1, f, idx], m16[:, :], AF.Ln)
            un = chpool.tile([128, ng, A], bf16, tag="u", name=f"un_{tag}")
            if lane == "D":
                r16 = chpool.tile([128, ng], f32, tag="r16", name=f"r16_{tag}")
                nc.vector.reciprocal(r16[:, :], m16[:, :])
                nc.vector.tensor_mul(un[:, :, :], u[:, :, :],
                                     r16[:, :, None].broadcast_to([128, ng, A]))
            else:
                nc.gpsimd.tensor_tensor(
                    un[:, :, :], u[:, :, :],
                    m16[:, :, None].broadcast_to([128, ng, A]), ALU.divide)
            state[("u", fold, lane)] = un

        def fold_step(fold, pos):
            # left: uL'[b] = sum_a uL[a]*P[a,b]  (P stored b-major)
            # right: uR'[a] = sum_b P[a,b]*uR[b] (P stored a-major)
            w, li = divmod(pos, 16)
            for lane, g0, g1 in LANES[fold]:
                ng = g1 - g0
                eng = nc.vector if lane == "D" else nc.gpsimd
                u = state[("u", fold, lane)]
                Pv = PP[w][:, g0:g1, li, :].rearrange(
                    "p g (o i) -> p g o i", o=A)
                ubc = u[:, :, None, :].broadcast_to([128, ng, A, A])
                tmp = chpool.tile([128, ng, A, A], bf16, tag="tmp" + lane,
                                  name=f"tmp{fold}{lane}{pos}")
                eng.tensor_mul(tmp[:, :, :, :], ubc, Pv)
                unew = chpool.tile([128, ng, A], bf16, tag="u" + lane,
                                   name=f"u{fold}{lane}{pos}")
                with nc.allow_low_precision(reason="renormalized chain fold"):
                    eng.tensor_reduce(unew[:, :, :], tmp[:, :, :, :],
                                      axis=AX.X, op=ALU.add)
                state[("u", fold, lane)] = unew
                # every 16 positions; worst measured 16-pos decay is e^-62,
                # far above the bf16 normal floor
                if (fold == "L" and li == 15) or (fold == "R" and li == 8):
                    renorm(fold, lane, g0, g1, f"{fold}{lane}{pos}")

        def init_fold(fold):
            # storage major-axis o is b for left / a for right; in both cases
            # the seed vector is the i=0 slice over o
            w, li = (0, 0) if fold == "L" else (7, 15)
            for lane, g0, g1 in LANES[fold]:
                ng = g1 - g0
                eng = nc.vector if lane == "D" else nc.gpsimd
                u0 = chpool.tile([128, ng, A], bf16, tag="u" + lane,
                                 name=f"u_init{fold}{lane}")
                eng.tensor_copy(
                    u0[:, :, :],
                    PP[w][:, g0:g1, li, :].rearrange(
                        "p g (o i) -> p g o i", o=A)[:, :, :, 0])
                state[("u", fold, lane)] = u0

        FRONTS = {}

        def logp_front(ps, qc, gp):
            # ps [128, 1024]: g=2gp in [0:512], g=2gp+1 in [512:1024]
            # layout per half: (16 pos)(2 t)(16 e)
            psv = ps[:, :].rearrange("p (g i t e) -> p g i t e", g=2, t=2, e=16)
            mu_ap = psv[:, :, :, 0, :]             # [128, 2, 16, 16]
            al_ap = psv[:, :, :, 1, :]
            et = ltmp.tile([128, 2, 16, 16], bf16, tag="et", name=f"et{qc}_{gp}")
            nc.scalar.activation(et[:, :, :, :], al_ap, AF.Exp, scale=-SDESC)
            dt_ = ltmp2.tile([128, 2, 16, 16], bf16, tag="dt", name=f"dt{qc}_{gp}")
            for h in range(2):
                g = 2 * gp + h
                xbc = xf[:, g, qc * 16:(qc + 1) * 16][:, :, None] \
                    .broadcast_to([128, 16, 16])
                nc.vector.scalar_tensor_tensor(
                    out=dt_[:, h, :, :], in0=mu_ap[:, h, :, :],
                    scalar=SDESC, in1=xbc,
                    op0=ALU.mult, op1=ALU.subtract)
            tt_ = ltmp2.tile([128, 2, 16, 16], bf16, tag="tt", name=f"tt{qc}_{gp}")
            nc.gpsimd.tensor_mul(tt_[:, :, :, :], dt_[:, :, :, :], et[:, :, :, :])
            sq = ltmp.tile([128, 2, 16, 16], bf16, tag="sq", name=f"sq{qc}_{gp}")
            nc.gpsimd.tensor_mul(sq[:, :, :, :], tt_[:, :, :, :], tt_[:, :, :, :])
            FRONTS.setdefault(qc, []).append((et, sq))

        def logp_back(qc):
            for gp, (et, sq) in enumerate(FRONTS.pop(qc)):
                p1 = ltmp2.tile([128, 2, 16, 16], bf16, tag="p1",
                                name=f"p1{qc}_{gp}")
                nc.scalar.activation(p1[:, :, :, :], sq[:, :, :, :], AF.Exp,
                                     scale=-0.5, bias=cnegc[:, :])
                nc.vector.tensor_mul(PP[qc][:, 2 * gp:2 * gp + 2, :, :],
                                     p1[:, :, :, :], et[:, :, :, :])

        def l4_chunk(qc, w3tiles):
            npq = _NP_4[qc]
            for gp in range(4):
                ps = psB.tile([128, 1024], f32, tag="psB", name=f"ps4_{qc}_{gp}")
                for kp in range(npq):
                    for h in range(2):
                        g = 2 * gp + h
                        nc.tensor.matmul(
                            ps[:, 512 * h:512 * (h + 1)],
                            A3[kp][:, :, g * 128:(g + 1) * 128],
                            w3tiles[kp],
                            start=(kp == 0), stop=False,
                            perf_mode=mybir.MatmulPerfMode.DoubleRow)
                c1v = cone8[0:1, :].rearrange("p (two h) -> p two h", two=2)
                b3v = b3sb[0:1, qc * 1024:(qc + 1) * 1024].rearrange(
                    "p (two n) -> p two n", two=2)
                for h in range(2):
                    nc.tensor.matmul(ps[:, 512 * h:512 * (h + 1)],
                                     c1v, b3v, start=False, stop=True,
                                     perf_mode=mybir.MatmulPerfMode.DoubleRow)
                logp_front(ps, qc, gp)

        def w3_views(qc):
            src = w3sbL if qc < 4 else w3sbR
            base = _OFF3[qc] - (0 if qc < 4 else _OFF3[4])
            return [
                src[:, base + kp * 1024:base + (kp + 1) * 1024].rearrange(
                    "p (two n) -> p two n", two=2)
                for kp in range(_NP_4[qc])
            ]

        psA = ctx.enter_context(tc.tile_pool(name="psA", bufs=2, space="PSUM"))
        psB = ctx.enter_context(tc.tile_pool(name="psB", bufs=2, space="PSUM"))

        # transpose x (PE transpose, cast to bf16 on copy-out)
        for g in range(NG):
            pst = psA.tile([128, 1024], f32, tag="psA", name=f"pst{g}")
            nc.tensor.transpose(pst[:, 0:128], xf[:, g, :], ident[:, :])
            nc.scalar.activation(xt[:, 0, g * 128:(g + 1) * 128],
                                 pst[:, 0:128], AF.Copy, scale=float(2 ** SAX))

        def relu_out(m, ps, Adst, layer):
            # Adst slot [128, 1024] <- relu of ps, scaled to h*2^SA fp8
            dst = Adst[m // 2][:, m % 2, :]
            eng = RELU_MAP_L[layer][m]
            scale = float(2.0 ** (SA - SW0 - SAX)) if layer == 0 \
                else float(2.0 ** (-SW))
            if eng == "A":
                bsb = (b0sb, b1sb, b2sb)[layer]
                nc.scalar.activation(dst, ps[:, :], AF.Relu,
                                     bias=bsb[:, m:m + 1], scale=scale)
            else:
                nc.vector.tensor_scalar(dst, ps[:, :], scale, 0.0,
                                        ALU.mult, ALU.max)

        def bias_mm(m, ps, layer):
            # K=1 matmul adding bias (PSUM domain) for non-ACT chunks
            if RELU_MAP_L[layer][m] == "A":
                return
            assert layer == 0
            brow = b0wsb
            j = _DCH.index(m)
            for h in range(2):
                nc.tensor.matmul(ps[:, 512 * h:512 * (h + 1)],
                                 brow[0:1, j * 128:(j + 1) * 128],
                                 onesbl[0:1, 512 * h:512 * (h + 1)],
                                 start=False, stop=True)

        w0v = w0sb[:, :].rearrange("p (two h) -> p two h", two=2)

        def l1_chunk(m):
            ps = psA.tile([128, 1024], f32, tag="psA", name=f"ps1_{m}")
            last = RELU_MAP_L[0][m] == "A"
            for n in range(2):
                nc.tensor.matmul(ps[:, n * 512:(n + 1) * 512],
                                 w0v[:, :, m * 128:(m + 1) * 128],
                                 xt[:, :, n * 512:(n + 1) * 512],
                                 start=True, stop=last,
                                 perf_mode=mybir.MatmulPerfMode.DoubleRow)
            bias_mm(m, ps, 0)
            relu_out(m, ps, A1, 0)

        def hidden_chunk(wsb, Ain, Aout, m, layer, tagp):
            npair = _NP_L[m]
            wt = wsb[:, _OFF1[m]:_OFF1[m + 1]]
            ps = psA.tile([128, 1024], f32, tag="psA", name=f"ps{tagp}_{m}")
            last = RELU_MAP_L[layer][m] == "A"
            for kp in range(npair):
                lhs = wt[:, kp * 256:(kp + 1) * 256].rearrange(
                    "p (two m) -> p two m", two=2)
                for n in range(2):
                    nc.tensor.matmul(ps[:, n * 512:(n + 1) * 512], lhs,
                                     Ain[kp][:, :, n * 512:(n + 1) * 512],
                                     start=(kp == 0),
                                     stop=(last and kp == npair - 1),
                                     perf_mode=mybir.MatmulPerfMode.DoubleRow)
            bias_mm(m, ps, layer)
            relu_out(m, ps, Aout, layer)

        # ---- wavefront: wave k = L1/L2/L3 chunks (2k, 2k+1), L4 chunk k,
        # fold block k.  The triangular masks make chunk m of each layer
        # depend only on pairs <= m//2 of the previous layer, so every layer
        # pipelines after just two chunks of its predecessor, and the serial
        # fold chain starts ~wave 0 instead of after L3.  Meet point at
        # position 112: left fold covers blocks 0-6 (rides the waves), the
        # right fold only block 7 after the last wave.
        A1 = [a13p.tile([128, 2, BL], f8, tag="a13", name=f"a1_{kp}")
              for kp in range(NK // 2)]
        A2 = [a2p.tile([128, 2, BL], f8, tag="a2", name=f"a2_{kp}")
              for kp in range(NK // 2)]
        A3 = [a3p.tile([128, 2, BL], f8, tag="a3", name=f"a3_{kp}")
              for kp in range(NK // 2)]
        # software-pipelined: iteration t emits L1 pair t, L2 pair t-1,
        # L3 pair t-2, L4 chunk t-3, fold block t-4 -- every cross-layer
        # dependency is a full iteration old, so the in-order engine streams
        # never stall on each other mid-iteration.
        for t in range(12):
            marker(f"q{t}")
            if t < 8:
                for mm in (2 * t, 2 * t + 1):
                    l1_chunk(mm)
            if 1 <= t < 9:
                for mm in (2 * (t - 1), 2 * (t - 1) + 1):
                    hidden_chunk(w1sb, A1, A2, mm, 1, "a2")
            if 2 <= t < 10:
                for mm in (2 * (t - 2), 2 * (t - 2) + 1):
                    hidden_chunk(w2sb, A2, A3, mm, 2, "a3")
            if 3 <= t < 11:
                l4_chunk(t - 3, w3_views(t - 3))
            if 4 <= t:
                kf = t - 4
                logp_back(kf)
                if kf == 0:
                    init_fold("L")
                    for pos in range(1, 16):
                        fold_step("L", pos)
                elif kf < 6:
                    for pos in range(16 * kf, 16 * (kf + 1)):
                        fold_step("L", pos)
        # zip the last left block with the right fold: two independent serial
        # chains interleaved on DVE fill each other's latency gaps
        marker("foldR")
        init_fold("R")
        for i in range(16):
            fold_step("L", 96 + i)
            if i < 15:
                fold_step("R", 126 - i)

        marker("final")
        # ---- finalize: res = ln(sum_a uL[a]*uR[a]) + sum(logs) ----
        uL = state[("u", "L", "D")]
        tot = chpool.tile([128, NG], f32, tag="tot")
        for lane, g0, g1 in LANES["R"]:
            ng = g1 - g0
            eng = nc.vector if lane == "D" else nc.gpsimd
            uR = state[("u", "R", lane)]
            tmp2 = chpool.tile([128, ng, A], f32, tag="tmp2" + lane,
                               name=f"tmp2{lane}")
            eng.tensor_mul(tmp2[:, :, :], uL[:, g0:g1, :], uR[:, :, :])
            eng.tensor_reduce(tot[:, g0:g1], tmp2[:, :, :], axis=AX.X,
                              op=ALU.add)
        lgt = chpool.tile([128, NG], f32, tag="lgt")
        nc.scalar.activation(lgt[:, :], tot[:, :], AF.Ln)
        ssum = chpool.tile([128, NG], f32, tag="ssum")
        nc.vector.tensor_reduce(ssum[:, :], logs[:, :, :, :], axis=AX.XY,
                                op=ALU.add)
        res = chpool.tile([128, NG], f32, tag="res")
        nc.vector.tensor_add(res[:, :], lgt[:, :], ssum[:, :])
        nc.sync.dma_start(out=out_d[:, :], in_=res[:, :])

    # Pin all activations to the one table set that covers Copy/Relu/Exp/Ln.
    import concourse.hw_specs as hw_specs
    _orig_tables = hw_specs.get_activation_tables(nc.m.arch)
    _pinned = {
        name: (funcs if name == "natural_log_exp_and_others" else set())
        for name, funcs in _orig_tables.items()
    }
    _orig_fn = hw_specs.get_activation_tables
    try:
        hw_specs.get_activation_tables = lambda arch: _pinned
        import concourse.bacc as bacc_mod
        if hasattr(bacc_mod, "get_activation_tables"):
            bacc_mod.get_activation_tables = lambda arch: _pinned
        nc.compile()
    finally:
        hw_specs.get_activation_tables = _orig_fn
        if hasattr(bacc_mod, "get_activation_tables"):
            bacc_mod.get_activation_tables = _orig_fn
    return nc


def _get_nc(loop_reps=1):
    key = ("nc", loop_reps)
    if key not in _NC_CACHE:
        _NC_CACHE[key] = _build_nc(loop_reps)
    return _NC_CACHE[key]


def run_on_hw(in_maps, trace=False):
    from concourse.bass_utils import run_bass_kernel_spmd
    nc = _get_nc()
    return run_bass_kernel_spmd(nc, in_maps, list(range(NCORES)), trace=trace)


def kernel(**inputs):
    inputs = {k: np.asarray(v, dtype=np.float32) for k, v in inputs.items()}
    in_maps = _prep_inputs(
        inputs["x"], inputs["W0"], inputs["b0"], inputs["W1"], inputs["b1"],
        inputs["W2"], inputs["b2"], inputs["W3"], inputs["b3"])
    res = run_on_hw(in_maps)
    out = np.empty((B,), np.float32)
    for c in range(NCORES):
        out[c * BL:(c + 1) * BL] = res.results[c]["out"].T.reshape(BL)
    return out.reshape(B, 1, 1)


# revision 51
# speedup vs baseline: 1.0055x; 1.0055x over previous
"""Trainium2 Bass kernel for the DART masked-MLP + log-semiring chain model.

Computes, for B=8192 samples distributed over 8 NeuronCores (1024 each):
  h1 = relu(x @ (m0*W0).T + b0)
  h2 = relu(h1 @ (m1*W1).T + b1)
  h3 = relu(h2 @ (m2*W2).T + b2)
  theta = (h3 @ (m3*W3).T + b3) -> (B, 128, 2, 4, 4) = (mu, alpha)
  logp  = -0.5*((x - mu)*exp(-alpha))**2 - alpha - 0.5*log(2pi) - log(4)
  out   = logexpmm(first, logexpmm(chain(inner), last))   # (B, 1, 1)

Device strategy (per core):
  - MADE masks premultiplied into the weights host-side; hidden units are
    degree-sorted so the masked weights become block lower triangular and
    ~44% of contraction chunks are skipped.  All four matmul layers run in
    fp8-e4m3 DoubleRow (0.5 PE cycles/row); L1 zero-pads the second
    contraction lane, and all biases are injected with K=1 fp8 DR matmuls
    against constant lanes (no separate bias adds on the vector engines).
  - Software-pipelined wavefront: iteration t emits L1 chunk-pair t, L2
    pair t-1, L3 pair t-2, L4 chunk t-3 matmuls + logp front (et/dt/tt/sq),
    logp back (p1/P) for t-4, and chain-fold block t-4.  Every cross-engine
    dependency is a full iteration old, so the in-order engine streams never
    stall on each other, and the serial fold chain starts ~wave 0.
  - The log-semiring product is folded in the LINEAR domain from both ends
    as vector folds (one DVE op pair per position, bf16, renormalized every
    16 positions with Ln bookkeeping on ACT off the critical path; the
    measured worst 16-position decay e^-62 is far above the bf16 floor).
    Meet point at position 112: the left fold (111 steps) rides the waves;
    only the 15-step right fold trails the last wave, zipped with the left
    fold's final block so the two serial chains hide each other's gaps.
  - Left-chunk W3/b3 columns are packed (pos,t,b,a)-transposed host-side so
    every logp stage and P store stays stride-1 packed (DVE 2x bf16 mode).
  - Engine assignment: relu + the two exps on ACT, dt/P/folds on DVE,
    tt/sq muls on Pool (gpsimd; it cannot read PSUM), matmuls on PE.
    Weights stream as a few 128-descriptor whole-block DMAs ordered by
    first use (per-chunk DMAs previously saturated HWDGE descriptor gen).
"""

import math

import numpy as np
import ml_dtypes

I = 128          # input size / positions
H = 2048         # hidden
A = 4            # alpha_dim
K = 2 * A * A    # 32 theta entries per position
B = 8192
NCORES = 8
BL = B // NCORES          # 1024 samples per core
NG = BL // 128            # 8 sample groups of 128
NK = H // 128             # 16 hidden chunks
NQ = (I * K) // 512       # 8 output q-chunks (512 wide = 16 positions)
C0 = 0.5 * math.log(2.0 * math.pi) + math.log(4.0)
SW = 13                   # weight scale 2^SW for fp8
SA = 5                    # activation scale 2^SA for fp8
SW0 = 11                  # L1 weight scale (|w0| <= 1/sqrt(128))
SAX = 5                   # x scale for the fp8 L1 input

# relu engine split per layer: "A" (ACT fused) or "D" (DVE tensor_scalar
# after a K=1 bias matmul; Pool cannot read PSUM).  L1's early chunks go to
# DVE, which idles during pipeline fill; L2/L3 relus stay on ACT.
RELU_MAP_L = [["A"] * 16, ["A"] * 16, ["A"] * 16]

_DCH = [m for m in range(NK) if RELU_MAP_L[0][m] == "D"]

_bf16 = ml_dtypes.bfloat16
_f8 = ml_dtypes.float8_e4m3


def _make_meta():
    hdeg = np.arange(H) % (I - 1)
    perm = np.argsort(hdeg, kind="stable")
    sdeg = hdeg[perm]
    km_l = []
    for m in range(NK):
        dhi = sdeg[128 * m + 127]
        km_l.append(max(k for k in range(NK) if sdeg[128 * k] <= dhi))
    km_4 = []
    for qc in range(NQ):
        dhi = 16 * qc + 15 - 1
        cands = [k for k in range(NK) if sdeg[128 * k] <= dhi]
        km_4.append(max(cands) if cands else -1)
    return perm, km_l, km_4


_PERM, _KM_L, _KM_4 = _make_meta()
_NP_L = [k // 2 + 1 for k in _KM_L]     # fp8 DoubleRow pair-chunks per m
_NP_4 = [k // 2 + 1 for k in _KM_4]     # per L4 q-chunk
_OFF1 = np.cumsum([0] + [p * 256 for p in _NP_L]).tolist()
_OFF3 = np.cumsum([0] + [p * 1024 for p in _NP_4]).tolist()


def _prep_inputs(x, W0, b0, W1, b1, W2, b2, W3, b3):
    """Host-side: premask, degree-sort, pack and cast the weights."""
    inp = np.arange(I)
    degrees = [inp] + [np.arange(H) % (I - 1) for _ in range(3)] + [np.arange(I) - 1]
    masks = [
        (d1[:, None] >= d0[None, :]).astype(np.float32)
        for d0, d1 in zip(degrees[:-1], degrees[1:])
    ]
    masks[-1] = np.repeat(masks[-1], K, axis=0)

    p = _PERM
    W0s = (masks[0] * W0)[p]
    b0s = b0[p]
    W1s = (masks[1] * W1)[p][:, p]
    b1s = b1[p]
    W2s = (masks[2] * W2)[p][:, p]
    b2s = b2[p]
    W3s = (masks[3] * W3)[:, p]

    # Left chunks (qc 0-3): transpose (a,b)->(b,a) within each position so P
    # lands b-major (fold-ready) with stride-1 stores.  e = i*32 + t*16 + a*4 + b.
    colperm = np.arange(I * K)
    e = colperm.reshape(I, 2, A, A)
    e_t = np.transpose(e, (0, 1, 3, 2)).reshape(I * K)
    left_cols = np.arange(0, 4 * 512)
    colperm[left_cols] = e_t[left_cols]
    W3p = W3s[colperm]
    b3p = b3[colperm]

    # L1 weights as fp8 DoubleRow pairs: lane 0 = w0*2^SW0, lane 1 = 0.
    # x is cast to fp8 (x*2^SAX) on chip; psum = h1pre * 2^(SW0+SAX).
    w0dr = np.zeros((I, 2, H), np.float32)
    w0dr[:, 0, :] = W0s.T * float(2 ** SW0)
    assert np.abs(w0dr).max() < 440.0
    w0t = np.ascontiguousarray(w0dr.reshape(I, 2 * H)).astype(_f8)

    def pack_pairs(WT, npairs, out_w, col_starts):
        cols = []
        for blk, (np_, c0) in enumerate(zip(npairs, col_starts)):
            for kp in range(np_):
                blkdat = np.stack(
                    [WT[256 * kp + 128 * par:256 * kp + 128 * (par + 1),
                        c0:c0 + out_w] for par in range(2)], axis=1)
                cols.append(blkdat.reshape(128, 2 * out_w))
        arr = np.concatenate(cols, axis=1) * float(2 ** SW)
        assert np.abs(arr).max() < 440.0, np.abs(arr).max()
        return np.ascontiguousarray(arr).astype(_f8)

    w1t = pack_pairs(W1s.T, _NP_L, 128, [128 * m for m in range(NK)])
    w2t = pack_pairs(W2s.T, _NP_L, 128, [128 * m for m in range(NK)])
    w3t = pack_pairs(W3p.T, _NP_4, 512, [512 * q for q in range(NQ)])

    # per-partition biases for ACT relu chunks (scaled 2^SA)
    b0r = np.ascontiguousarray(b0s.reshape(NK, 128).T
                               ).astype(np.float32) * float(2 ** SA)
    b0q = np.ascontiguousarray(b0s.reshape(NK, 128).T
                               ).astype(np.float32) * float(2 ** (SW0 + SAX))
    b1r = np.ascontiguousarray(b1s.reshape(NK, 128).T
                               ).astype(np.float32) * float(2 ** SA)
    b2r = np.ascontiguousarray(b2s.reshape(NK, 128).T
                               ).astype(np.float32) * float(2 ** SA)
    # bias rows for DVE relu chunks (bias-matmul into PSUM domain), packed
    # to just the D chunks' columns
    if _DCH:
        dsel = np.concatenate([np.arange(128 * m, 128 * (m + 1))
                               for m in _DCH])
        b0w = (b0s[None, dsel] * float(2 ** (SW0 + SAX))).astype(_bf16)
    else:
        b0w = np.zeros((1, 1), np.float32).astype(_bf16)
    b1w = np.zeros((1, 1), np.float32).astype(_bf16)                  # unused
    b2w = np.zeros((1, 1), np.float32).astype(_bf16)                  # unused
    # L4 bias as fp8 K=1 DoubleRow rows: per chunk [1024] = (b3*2^SW | zeros);
    # the stationary constant lane supplies 2^SA, so psum += b3*2^(SW+SA).
    b3blk = np.zeros((NQ, 2, 512), np.float32)
    b3blk[:, 0, :] = b3p.reshape(NQ, 512) * float(2 ** SW)
    assert np.abs(b3blk).max() < 440.0
    b3r = np.ascontiguousarray(b3blk.reshape(1, NQ * 1024)).astype(_f8)

    common = dict(w0t=w0t, w1t=w1t, w2t=w2t, w3t=w3t,
                  b0r=b0r, b0q=b0q, b1r=b1r, b2r=b2r,
                  b0w=b0w, b1w=b1w, b2w=b2w, b3r=b3r)
    in_maps = []
    for c in range(NCORES):
        m = dict(common)
        m["x"] = np.ascontiguousarray(x[c * BL:(c + 1) * BL]).astype(np.float32)
        in_maps.append(m)
    return in_maps


_NC_CACHE = {}
_MARKERS = {}


def _patch_ldw_opt():
    """Enable walrus's ldweights overlap: without it every matmul pays a
    serial ~107ns stationary load (measured ~310ns/DoubleRow-matmul vs the
    ~107ns model), which makes instruction count the real PE wall."""
    import concourse.bass_utils as _bu
    if getattr(_bu, "_ldw_patched", False):
        return
    _orig = _bu.run_command

    def _patched(cmd, *a, **k):
        if isinstance(cmd, list):
            cmd = ["--enable-ldw-opt=true" if c == "--enable-ldw-opt=false"
                   else c for c in cmd]
        return _orig(cmd, *a, **k)

    _bu.run_command = _patched
    _bu._ldw_patched = True


def _build_nc(loop_reps=1):
    import concourse.bacc as bacc
    import concourse.tile as tile
    import concourse.mybir as mybir
    from concourse.masks import make_identity
    from contextlib import ExitStack

    f32 = mybir.dt.float32
    bf16 = mybir.dt.bfloat16
    f8 = mybir.dt.float8e4
    AF = mybir.ActivationFunctionType
    ALU = mybir.AluOpType
    AX = mybir.AxisListType

    nc = bacc.Bacc("TRN2")
    x_d = nc.declare_dram_parameter("x", [BL, I], f32, isOutput=False)
    w0_d = nc.declare_dram_parameter("w0t", [I, 2 * H], f8, isOutput=False)
    w1_d = nc.declare_dram_parameter("w1t", [128, _OFF1[-1]], f8, isOutput=False)
    w2_d = nc.declare_dram_parameter("w2t", [128, _OFF1[-1]], f8, isOutput=False)
    w3_d = nc.declare_dram_parameter("w3t", [128, _OFF3[-1]], f8, isOutput=False)
    b0_d = nc.declare_dram_parameter("b0r", [128, NK], f32, isOutput=False)
    b0q_d = nc.declare_dram_parameter("b0q", [128, NK], f32, isOutput=False)
    b1_d = nc.declare_dram_parameter("b1r", [128, NK], f32, isOutput=False)
    b2_d = nc.declare_dram_parameter("b2r", [128, NK], f32, isOutput=False)
    b0w_d = nc.declare_dram_parameter("b0w", [1, max(1, len(_DCH) * 128)], bf16, isOutput=False)
    b1w_d = nc.declare_dram_parameter("b1w", [1, 1], bf16, isOutput=False)
    b2w_d = nc.declare_dram_parameter("b2w", [1, 1], bf16, isOutput=False)
    b3_d = nc.declare_dram_parameter("b3r", [1, NQ * 1024], f8, isOutput=False)
    out_d = nc.declare_dram_parameter("out", [128, NG], f32, isOutput=True)

    SDESC = float(2.0 ** (-(SW + SA)))

    with ExitStack() as ctx:
        tc = ctx.enter_context(tile.TileContext(nc))
        consts = ctx.enter_context(tc.tile_pool(name="consts", bufs=1))
        a13p = ctx.enter_context(tc.tile_pool(name="a13p", bufs=NK // 2))
        a2p = ctx.enter_context(tc.tile_pool(name="a2p", bufs=NK // 2))
        a3p = ctx.enter_context(tc.tile_pool(name="a3p", bufs=NK // 2))
        pallpool = ctx.enter_context(tc.tile_pool(name="pallp", bufs=8))
        ltmp = ctx.enter_context(tc.tile_pool(name="ltmp", bufs=8))
        ltmp2 = ctx.enter_context(tc.tile_pool(name="ltmp2", bufs=4))
        chpool = ctx.enter_context(tc.tile_pool(name="ch", bufs=4))

        if loop_reps > 1:
            ctx.enter_context(tc.For_i(0, loop_reps, 1))

        def marker(tag):
            _MARKERS[tag] = int(nc.get_next_instruction_name()[2:])

        # ---- constants ----
        xf = consts.tile([128, NG, I], f32)         # x[p, g, i] = x[g*128+p, i]
        nc.sync.dma_start(out=xf, in_=x_d[:, :].rearrange("(g p) i -> p g i", p=128))
        w0sb = consts.tile([128, 2 * H], f8)
        nc.sync.dma_start(out=w0sb, in_=w0_d[:, :])
        b0sb = consts.tile([128, NK], f32)
        nc.sync.dma_start(out=b0sb, in_=b0_d[:, :])
        b0qsb = consts.tile([128, NK], f32)
        nc.sync.dma_start(out=b0qsb, in_=b0q_d[:, :])
        b1sb = consts.tile([128, NK], f32)
        nc.sync.dma_start(out=b1sb, in_=b1_d[:, :])
        b2sb = consts.tile([128, NK], f32)
        nc.sync.dma_start(out=b2sb, in_=b2_d[:, :])
        b3sb = consts.tile([1, NQ * 1024], f8)
        nc.sync.dma_start(out=b3sb, in_=b3_d[:, :])
        cone8 = consts.tile([1, 256], f8)
        nc.vector.memset(cone8, 0.0)
        nc.vector.memset(cone8[0:1, 0:128], float(2 ** SA))
        b0wsb = consts.tile([1, max(1, len(_DCH) * 128)], bf16)
        nc.sync.dma_start(out=b0wsb, in_=b0w_d[:, :])

        onesbl = consts.tile([1, BL], bf16)
        nc.vector.memset(onesbl, 1.0)
        # weight loads in a few large pieces (128 descriptors each), ordered
        # by first use so compute starts as soon as its piece lands
        w1sb = consts.tile([128, _OFF1[-1]], f8)
        w2sb = consts.tile([128, _OFF1[-1]], f8)
        w3sbL = consts.tile([128, _OFF3[4]], f8)
        w3sbR = consts.tile([128, _OFF3[-1] - _OFF3[4]], f8)
        for k in range(NK // 2):
            nc.sync.dma_start(out=w1sb[:, _OFF1[2 * k]:_OFF1[2 * k + 2]],
                              in_=w1_d[:, _OFF1[2 * k]:_OFF1[2 * k + 2]])
            nc.sync.dma_start(out=w2sb[:, _OFF1[2 * k]:_OFF1[2 * k + 2]],
                              in_=w2_d[:, _OFF1[2 * k]:_OFF1[2 * k + 2]])
            if k < 4:
                nc.sync.dma_start(out=w3sbL[:, _OFF3[k]:_OFF3[k + 1]],
                                  in_=w3_d[:, _OFF3[k]:_OFF3[k + 1]])
            else:
                nc.sync.dma_start(
                    out=w3sbR[:, _OFF3[k] - _OFF3[4]:_OFF3[k + 1] - _OFF3[4]],
                    in_=w3_d[:, _OFF3[k]:_OFF3[k + 1]])
        ones1 = consts.tile([1, 128], bf16)
        nc.vector.memset(ones1, 1.0)
        czero = consts.tile([128, 1], f32)
        nc.vector.memset(czero, 0.0)
        nc.const_aps.aps[(f32, 0.0)] = czero[:, :]
        cnegc = consts.tile([128, 1], f32)
        nc.vector.memset(cnegc, -C0)
        ident = consts.tile([128, 128], f32)
        make_identity(nc, ident[:, :])
        NREN = 16
        logs = consts.tile([128, NG, 2, NREN], f32)
        nc.vector.memset(logs, 0.0)
        xt = consts.tile([128, 2, BL], f8)          # xT[i, lane, g*128+b]
        nc.vector.memset(xt, 0.0)

        # P tiles: [128, g, pos, 16e]; left (w 0-3) b-major, right (w 4-7)
        # a-major.  bf16.
        PP = {}
        for w in range(8):
            PP[w] = pallpool.tile([128, NG, 16, 16], bf16, tag="pall",
                                  name=f"pp_{w}")

        # both folds run whole-NG on DVE (Pool free-axis reduce unsupported);
        # the lane machinery is kept for experiments
        LANES = {"L": [("D", 0, NG)], "R": [("D", 0, NG)]}
        state = {("u", f, ln): None for f in "LR" for ln, _, _ in LANES[f]}
        state.update({("n", f, ln): 0 for f in "LR" for ln, _, _ in LANES[f]})

        def renorm(fold, lane, g0, g1, tag):
            # rescale fold vector so max=1; Ln bookkeeping on ACT (parallel)
            f = 0 if fold == "L" else 1
            idx = state[("n", fold, lane)]
            state[("n", fold, lane)] += 1
            u = state[("u", fold, lane)]
            ng = g1 - g0
            eng = nc.vector if lane == "D" else nc.gpsimd
            m16 = chpool.tile([128, ng], f32, tag="m16", name=f"m16_{tag}")
            eng.tensor_reduce(m16[:, :], u[:, :, :], axis=AX.X, op=ALU.max)
            nc.scalar.activation(logs[:, g0:g1, f, idx], m16[:, :], AF.Ln)
            un = chpool.tile([128, ng, A], bf16, tag="u", name=f"un_{tag}")
            if lane == "D":
                r16 = chpool.tile([128, ng], f32, tag="r16", name=f"r16_{tag}")
                nc.vector.reciprocal(r16[:, :], m16[:, :])
                nc.vector.tensor_mul(un[:, :, :], u[:, :, :],
                                     r16[:, :, None].broadcast_to([128, ng, A]))
            else:
                nc.gpsimd.tensor_tensor(
                    un[:, :, :], u[:, :, :],
                    m16[:, :, None].broadcast_to([128, ng, A]), ALU.divide)
            state[("u", fold, lane)] = un

        def fold_step(fold, pos):
            # left: uL'[b] = sum_a uL[a]*P[a,b]  (P stored b-major)
            # right: uR'[a] = sum_b P[a,b]*uR[b] (P stored a-major)
            w, li = divmod(pos, 16)
            for lane, g0, g1 in LANES[fold]:
                ng = g1 - g0
                eng = nc.vector if lane == "D" else nc.gpsimd
                u = state[("u", fold, lane)]
                Pv = PP[w][:, g0:g1, li, :].rearrange(
                    "p g (o i) -> p g o i", o=A)
                ubc = u[:, :, None, :].broadcast_to([128, ng, A, A])
                tmp = chpool.tile([128, ng, A, A], bf16, tag="tmp" + lane,
                                  name=f"tmp{fold}{lane}{pos}")
                eng.tensor_mul(tmp[:, :, :, :], ubc, Pv)
                unew = chpool.tile([128, ng, A], bf16, tag="u" + lane,
                                   name=f"u{fold}{lane}{pos}")
                with nc.allow_low_precision(reason="renormalized chain fold"):
                    eng.tensor_reduce(unew[:, :, :], tmp[:, :, :, :],
                                      axis=AX.X, op=ALU.add)
                state[("u", fold, lane)] = unew
                # every 16 positions; worst measured 16-pos decay is e^-62,
                # far above the bf16 normal floor
                if (fold == "L" and li == 15) or (fold == "R" and li == 8):
                    renorm(fold, lane, g0, g1, f"{fold}{lane}{pos}")

        def init_fold(fold):
            # storage major-axis o is b for left / a for right; in both cases
            # the seed vector is the i=0 slice over o
            w, li = (0, 0) if fold == "L" else (7, 15)
            for lane, g0, g1 in LANES[fold]:
                ng = g1 - g0
                eng = nc.vector if lane == "D" else nc.gpsimd
                u0 = chpool.tile([128, ng, A], bf16, tag="u" + lane,
                                 name=f"u_init{fold}{lane}")
                eng.tensor_copy(
                    u0[:, :, :],
                    PP[w][:, g0:g1, li, :].rearrange(
                        "p g (o i) -> p g o i", o=A)[:, :, :, 0])
                state[("u", fold, lane)] = u0

        FRONTS = {}

        def logp_front(ps, qc, gp):
            # ps [128, 1024]: g=2gp in [0:512], g=2gp+1 in [512:1024]
            # layout per half: (16 pos)(2 t)(16 e)
            psv = ps[:, :].rearrange("p (g i t e) -> p g i t e", g=2, t=2, e=16)
            mu_ap = psv[:, :, :, 0, :]             # [128, 2, 16, 16]
            al_ap = psv[:, :, :, 1, :]
            et = ltmp.tile([128, 2, 16, 16], bf16, tag="et", name=f"et{qc}_{gp}")
            nc.scalar.activation(et[:, :, :, :], al_ap, AF.Exp, scale=-SDESC)
            dt_ = ltmp2.tile([128, 2, 16, 16], bf16, tag="dt", name=f"dt{qc}_{gp}")
            for h in range(2):
                g = 2 * gp + h
                xbc = xf[:, g, qc * 16:(qc + 1) * 16][:, :, None] \
                    .broadcast_to([128, 16, 16])
                nc.vector.scalar_tensor_tensor(
                    out=dt_[:, h, :, :], in0=mu_ap[:, h, :, :],
                    scalar=SDESC, in1=xbc,
                    op0=ALU.mult, op1=ALU.subtract)
            tt_ = ltmp2.tile([128, 2, 16, 16], bf16, tag="tt", name=f"tt{qc}_{gp}")
            nc.gpsimd.tensor_mul(tt_[:, :, :, :], dt_[:, :, :, :], et[:, :, :, :])
            sq = ltmp.tile([128, 2, 16, 16], bf16, tag="sq", name=f"sq{qc}_{gp}")
            nc.gpsimd.tensor_mul(sq[:, :, :, :], tt_[:, :, :, :], tt_[:, :, :, :])
            FRONTS.setdefault(qc, []).append((et, sq))

        def logp_back(qc):
            for gp, (et, sq) in enumerate(FRONTS.pop(qc)):
                p1 = ltmp2.tile([128, 2, 16, 16], bf16, tag="p1",
                                name=f"p1{qc}_{gp}")
                nc.scalar.activation(p1[:, :, :, :], sq[:, :, :, :], AF.Exp,
                                     scale=-0.5, bias=cnegc[:, :])
                nc.vector.tensor_mul(PP[qc][:, 2 * gp:2 * gp + 2, :, :],
                                     p1[:, :, :, :], et[:, :, :, :])

        def l4_chunk(qc, w3tiles):
            npq = _NP_4[qc]
            for gp in range(4):
                ps = psB.tile([128, 1024], f32, tag="psB", name=f"ps4_{qc}_{gp}")
                for kp in range(npq):
                    for h in range(2):
                        g = 2 * gp + h
                        nc.tensor.matmul(
                            ps[:, 512 * h:512 * (h + 1)],
                            A3[kp][:, :, g * 128:(g + 1) * 128],
                            w3tiles[kp],
                            start=(kp == 0), stop=False,
                            perf_mode=mybir.MatmulPerfMode.DoubleRow)
                c1v = cone8[0:1, :].rearrange("p (two h) -> p two h", two=2)
                b3v = b3sb[0:1, qc * 1024:(qc + 1) * 1024].rearrange(
                    "p (two n) -> p two n", two=2)
                for h in range(2):
                    nc.tensor.matmul(ps[:, 512 * h:512 * (h + 1)],
                                     c1v, b3v, start=False, stop=True,
                                     perf_mode=mybir.MatmulPerfMode.DoubleRow)
                logp_front(ps, qc, gp)

        def w3_views(qc):
            src = w3sbL if qc < 4 else w3sbR
            base = _OFF3[qc] - (0 if qc < 4 else _OFF3[4])
            return [
                src[:, base + kp * 1024:base + (kp + 1) * 1024].rearrange(
                    "p (two n) -> p two n", two=2)
                for kp in range(_NP_4[qc])
            ]

        psA = ctx.enter_context(tc.tile_pool(name="psA", bufs=2, space="PSUM"))
        psB = ctx.enter_context(tc.tile_pool(name="psB", bufs=2, space="PSUM"))

        # transpose x (PE transpose, cast to bf16 on copy-out)
        for g in range(NG):
            pst = psA.tile([128, 1024], f32, tag="psA", name=f"pst{g}")
            nc.tensor.transpose(pst[:, 0:128], xf[:, g, :], ident[:, :])
            nc.scalar.activation(xt[:, 0, g * 128:(g + 1) * 128],
                                 pst[:, 0:128], AF.Copy, scale=float(2 ** SAX))

        def relu_out(m, ps, Adst, layer):
            # Adst slot [128, 1024] <- relu of ps, scaled to h*2^SA fp8
            dst = Adst[m // 2][:, m % 2, :]
            eng = RELU_MAP_L[layer][m]
            scale = float(2.0 ** (SA - SW0 - SAX)) if layer == 0 \
                else float(2.0 ** (-SW))
            if eng == "A":
                bsb = (b0sb, b1sb, b2sb)[layer]
                nc.scalar.activation(dst, ps[:, :], AF.Relu,
                                     bias=bsb[:, m:m + 1], scale=scale)
            else:
                nc.vector.tensor_scalar(dst, ps[:, :], scale, 0.0,
                                        ALU.mult, ALU.max)

        def bias_mm(m, ps, layer):
            # K=1 matmul adding bias (PSUM domain) for non-ACT chunks
            if RELU_MAP_L[layer][m] == "A":
                return
            assert layer == 0
            brow = b0wsb
            j = _DCH.index(m)
            for h in range(2):
                nc.tensor.matmul(ps[:, 512 * h:512 * (h + 1)],
                                 brow[0:1, j * 128:(j + 1) * 128],
                                 onesbl[0:1, 512 * h:512 * (h + 1)],
                                 start=False, stop=True)

        w0v = w0sb[:, :].rearrange("p (two h) -> p two h", two=2)

        def l1_chunk(m):
            ps = psA.tile([128, 1024], f32, tag="psA", name=f"ps1_{m}")
            last = RELU_MAP_L[0][m] == "A"
            for n in range(2):
                nc.tensor.matmul(ps[:, n * 512:(n + 1) * 512],
                                 w0v[:, :, m * 128:(m + 1) * 128],
                                 xt[:, :, n * 512:(n + 1) * 512],
                                 start=True, stop=last,
                                 perf_mode=mybir.MatmulPerfMode.DoubleRow)
            bias_mm(m, ps, 0)
            relu_out(m, ps, A1, 0)

        def hidden_chunk(wsb, Ain, Aout, m, layer, tagp):
            npair = _NP_L[m]
            wt = wsb[:, _OFF1[m]:_OFF1[m + 1]]
            ps = psA.tile([128, 1024], f32, tag="psA", name=f"ps{tagp}_{m}")
            last = RELU_MAP_L[layer][m] == "A"
            for kp in range(npair):
                lhs = wt[:, kp * 256:(kp + 1) * 256].rearrange(
                    "p (two m) -> p two m", two=2)
                for n in range(2):
                    nc.tensor.matmul(ps[:, n * 512:(n + 1) * 512], lhs,
                                     Ain[kp][:, :, n * 512:(n + 1) * 512],
                                     start=(kp == 0),
                                     stop=(last and kp == npair - 1),
                                     perf_mode=mybir.MatmulPerfMode.DoubleRow)
            bias_mm(m, ps, layer)
            relu_out(m, ps, Aout, layer)

        # ---- wavefront: wave k = L1/L2/L3 chunks (2k, 2k+1), L4 chunk k,
        # fold block k.  The triangular masks make chunk m of each layer
        # depend only on pairs <= m//2 of the previous layer, so every layer
        # pipelines after just two chunks of its predecessor, and the serial
        # fold chain starts ~wave 0 instead of after L3.  Meet point at
        # position 112: left fold covers blocks 0-6 (rides the waves), the
        # right fold only block 7 after the last wave.
        A1 = [a13p.tile([128, 2, BL], f8, tag="a13", name=f"a1_{kp}")
              for kp in range(NK // 2)]
        A2 = [a2p.tile([128, 2, BL], f8, tag="a2", name=f"a2_{kp}")
              for kp in range(NK // 2)]
        A3 = [a3p.tile([128, 2, BL], f8, tag="a3", name=f"a3_{kp}")
              for kp in range(NK // 2)]
        # software-pipelined: iteration t emits L1 pair t, L2 pair t-1,
        # L3 pair t-2, L4 chunk t-3, fold block t-4 -- every cross-layer
        # dependency is a full iteration old, so the in-order engine streams
        # never stall on each other mid-iteration.
        for t in range(12):
            marker(f"q{t}")
            # L4 first: its matmuls drain into the psB pool, giving the
            # previous iteration's relus time to free psA buffers before
            # L1/L2/L3 need them (PE otherwise stalls on psum recycling)
            if 3 <= t < 11:
                l4_chunk(t - 3, w3_views(t - 3))
            if t < 8:
                for mm in (2 * t, 2 * t + 1):
                    l1_chunk(mm)
            if 1 <= t < 9:
                for mm in (2 * (t - 1), 2 * (t - 1) + 1):
                    hidden_chunk(w1sb, A1, A2, mm, 1, "a2")
            if 2 <= t < 10:
                for mm in (2 * (t - 2), 2 * (t - 2) + 1):
                    hidden_chunk(w2sb, A2, A3, mm, 2, "a3")
            if 4 <= t:
                kf = t - 4
                logp_back(kf)
                if kf == 0:
                    init_fold("L")
                    for pos in range(1, 16):
                        fold_step("L", pos)
                elif kf < 6:
                    for pos in range(16 * kf, 16 * (kf + 1)):
                        fold_step("L", pos)
        # zip the last left block with the right fold: two independent serial
        # chains interleaved on DVE fill each other's latency gaps
        marker("foldR")
        init_fold("R")
        for i in range(16):
            fold_step("L", 96 + i)
            if i < 15:
                fold_step("R", 126 - i)

        marker("final")
        # ---- finalize: res = ln(sum_a uL[a]*uR[a]) + sum(logs) ----
        uL = state[("u", "L", "D")]
        tot = chpool.tile([128, NG], f32, tag="tot")
        for lane, g0, g1 in LANES["R"]:
            ng = g1 - g0
            eng = nc.vector if lane == "D" else nc.gpsimd
            uR = state[("u", "R", lane)]
            tmp2 = chpool.tile([128, ng, A], f32, tag="tmp2" + lane,
                               name=f"tmp2{lane}")
            eng.tensor_mul(tmp2[:, :, :], uL[:, g0:g1, :], uR[:, :, :])
            eng.tensor_reduce(tot[:, g0:g1], tmp2[:, :, :], axis=AX.X,
                              op=ALU.add)
        lgt = chpool.tile([128, NG], f32, tag="lgt")
        nc.scalar.activation(lgt[:, :], tot[:, :], AF.Ln)
        ssum = chpool.tile([128, NG], f32, tag="ssum")
        nc.vector.tensor_reduce(ssum[:, :], logs[:, :, :, :], axis=AX.XY,
                                op=ALU.add)
        res = chpool.tile([128, NG], f32, tag="res")
        nc.vector.tensor_add(res[:, :], lgt[:, :], ssum[:, :])
        nc.sync.dma_start(out=out_d[:, :], in_=res[:, :])

    # Pin all activations to the one table set that covers Copy/Relu/Exp/Ln.
    import concourse.hw_specs as hw_specs
    _orig_tables = hw_specs.get_activation_tables(nc.m.arch)
    _pinned = {
        name: (funcs if name == "natural_log_exp_and_others" else set())
        for name, funcs in _orig_tables.items()
    }
    _orig_fn = hw_specs.get_activation_tables
    try:
        hw_specs.get_activation_tables = lambda arch: _pinned
        import concourse.bacc as bacc_mod
        if hasattr(bacc_mod, "get_activation_tables"):
            bacc_mod.get_activation_tables = lambda arch: _pinned
        nc.compile()
    finally:
        hw_specs.get_activation_tables = _orig_fn
        if hasattr(bacc_mod, "get_activation_tables"):
            bacc_mod.get_activation_tables = _orig_fn
    return nc


def _get_nc(loop_reps=1):
    key = ("nc", loop_reps)
    if key not in _NC_CACHE:
        _NC_CACHE[key] = _build_nc(loop_reps)
    return _NC_CACHE[key]


def run_on_hw(in_maps, trace=False):
    from concourse.bass_utils import run_bass_kernel_spmd
    nc = _get_nc()
    return run_bass_kernel_spmd(nc, in_maps, list(range(NCORES)), trace=trace)


def kernel(**inputs):
    inputs = {k: np.asarray(v, dtype=np.float32) for k, v in inputs.items()}
    in_maps = _prep_inputs(
        inputs["x"], inputs["W0"], inputs["b0"], inputs["W1"], inputs["b1"],
        inputs["W2"], inputs["b2"], inputs["W3"], inputs["b3"])
    res = run_on_hw(in_maps)
    out = np.empty((B,), np.float32)
    for c in range(NCORES):
        out[c * BL:(c + 1) * BL] = res.results[c]["out"].T.reshape(BL)
    return out.reshape(B, 1, 1)
